# revision 20
# baseline (speedup 1.0000x reference)
"""Multi-head self-attention (RoPE, causal) Trainium2 kernel, v3.

Tensor-parallel over heads: 16 heads / 8 cores = 2 heads per core
(Megatron-style: Wq/Wk/Wv sharded on output dim, Wo on input dim).
Each core computes a full [S, D] partial of the output projection;
the host sums the 8 partials.

v3 changes vs v2:
- PV orientation flipped: stationary = [vA|1A|0] / [1B|0|vB] zero-padded
  128-col blocks (per k-tile), moving = the exp tile (512 q cols). Kills
  the LDWEIGHTS-bound PV of v2 (~90us -> ~61us PE) and produces pv in
  [d, q] layout with softmax denominators embedded at partitions 64 (A)
  and 0 (B) -- no ones-columns in the moving operand, no PV psum
  zero-matmuls, no attention transposes.
- v projection emitted directly in [k, d] layout (stationary = x tile,
  moving = Wv) -- kills the v transposes.
- Normalization: reciprocal of den rows -> tiny ones-stationary matmul
  broadcasts them across partitions -> one tensor_tensor per head per
  chunk normalizes pv before a single-stationary out-projection.
- One ACTIVATE per tile-instance (diag tiles exp a harmless garbage
  region that PV never reads).
- fp16 RoPE datapath and fp16 trig tables (halves trig DMA).
"""

import numpy as np

S = 4096
D = 1024
DK = 64
NCORES = 8
THETA = 10000.0
CH = 512          # sequence chunk (scores/PV moving free dim)
NCH = S // CH     # 8 chunks
VS = 256          # v_sb cols per k-tile: [vA|1A|0(63)] [1B|0(63)|vB]

_CACHE = {}


# ---------------------------------------------------------------------------
# host-side layout helpers
# ---------------------------------------------------------------------------

def _rope_perm64():
    """Permutation of a head's 64 dims so RoPE pairs line up for a
    32-lane stream_shuffle: quadrant q (32 partitions) holds pairs
    16q..16q+15 as [evens(16) | odds(16)]."""
    perm = np.zeros(64, np.int64)
    for d in range(64):
        j, odd = d // 2, d % 2
        pos = 32 * (j // 16) + 16 * odd + (j % 16)
        perm[pos] = d
    return perm


def _trig_tables():
    # partition p: pair index = 16*((p//32)%2) + p%16 ; odd slot if p%32 >= 16
    p = np.arange(128)
    pair = 16 * ((p // 32) % 2) + (p % 16)
    odd = (p % 32) >= 16
    inv_freq = THETA ** (-2.0 * pair / DK)           # [128]
    pos = np.arange(S, dtype=np.float64)
    ang = pos[None, :] * inv_freq[:, None]           # [128, S]
    cos = np.cos(ang).astype(np.float16)
    sin = (np.where(odd[:, None], 1.0, -1.0) * np.sin(ang)).astype(np.float16)
    return cos, sin


def _host_prep(x, Wq, Wk, Wv, Wo):
    x = np.asarray(x, dtype=np.float32).reshape(S, D)
    Wq = np.asarray(Wq, dtype=np.float32)
    Wk = np.asarray(Wk, dtype=np.float32)
    Wv = np.asarray(Wv, dtype=np.float32)
    Wo = np.asarray(Wo, dtype=np.float32)

    xT = np.ascontiguousarray(x.T).astype(np.float16)          # [D, S]
    cos, sin = _trig_tables()
    tri = (np.arange(128)[None, :] >= np.arange(128)[:, None])
    tri = tri.astype(np.float16)

    # den-broadcast selector matmul weights [128, 256]:
    # A-part [:, 0:128]: row 64 (denA) -> out partitions 0:64;
    # B-part [:, 128:256]: row 0 (denB) -> out partitions 64:128.
    bcw = np.zeros((128, 256), np.float32)
    bcw[64, 0:64] = 1.0
    bcw[0, 128 + 64:256] = 1.0
    bcw = bcw.astype(np.float16)

    perm = _rope_perm64()
    in_maps = []
    for c in range(NCORES):
        hA, hB = 2 * c, 2 * c + 1
        rows_qk = np.concatenate([64 * hA + perm, 64 * hB + perm])
        rows_v = np.arange(128 * c, 128 * c + 128)
        wq_c = np.ascontiguousarray(Wq[rows_qk, :].T).astype(np.float16)
        wk_c = np.ascontiguousarray(Wk[rows_qk, :].T).astype(np.float16)
        wv_c = np.ascontiguousarray(Wv[rows_v, :].T).astype(np.float16)
        wo_c = np.ascontiguousarray(Wo[:, rows_v].T).astype(np.float16)
        in_maps.append({
            "xT": xT, "wq": wq_c, "wk": wk_c, "wv": wv_c, "wo": wo_c,
            "cos": cos, "sin": sin, "tri": tri, "bcw": bcw,
            "ones": np.ones((128, 1), np.float16),
        })
    return in_maps


# ---------------------------------------------------------------------------
# device program
# ---------------------------------------------------------------------------

def _emit(tc, out, xT, wq, wk, wv, wo, cos, sin, tri, bcw, ones, nch=NCH):
    import concourse.mybir as mybir

    nc = tc.nc
    f32 = mybir.dt.float32
    f16 = mybir.dt.float16
    AF = mybir.ActivationFunctionType
    OP = mybir.AluOpType
    SWAP_MASK = [(i + 16) % 32 for i in range(32)]

    with (
        tc.tile_pool(name="consts", bufs=1) as consts,
        tc.tile_pool(name="persist", bufs=1) as persist,
        tc.tile_pool(name="xtp", bufs=2) as xtp,
        tc.tile_pool(name="rope", bufs=2) as ropep,
        tc.tile_pool(name="trig", bufs=2) as trigp,
        tc.tile_pool(name="expp", bufs=4) as expp,
        tc.tile_pool(name="small", bufs=4) as smallp,
        tc.tile_pool(name="pvsb", bufs=2) as pvsbp,
        tc.tile_pool(name="bcp", bufs=2) as bcp,
        tc.tile_pool(name="stagep", bufs=3) as stagep,
        tc.tile_pool(name="ps_s", bufs=2, space="PSUM") as ps_scores,
        tc.tile_pool(name="ps_pv", bufs=1, space="PSUM") as ps_pv,
        tc.tile_pool(name="ps_o", bufs=1, space="PSUM") as ps_out,
        tc.tile_pool(name="ps_m", bufs=1, space="PSUM") as ps_misc,
    ):
        pstate = {}

        def fetch_chunk(j):
            """Issue chunk j's x and trig DMAs."""
            if ("xt", j) in pstate:
                return
            jsl = slice(j * CH, (j + 1) * CH)
            xt = xtp.tile([128, 8 * CH], f16, tag="xt")
            pstate[("xt", j)] = xt
            nc.sync.dma_start(
                out=xt.rearrange("p (t s) -> p t s", s=CH),
                in_=xT[:, jsl].rearrange("(t p) s -> p t s", p=128),
            )
            cs = trigp.tile([128, CH], f16, tag="cs")
            nc.sync.dma_start(out=cs, in_=cos[:, jsl])
            sn = trigp.tile([128, CH], f16, tag="sn")
            nc.sync.dma_start(out=sn, in_=sin[:, jsl])
            pstate[("trig", j)] = (cs, sn)

        # ---- constants (critical-path DMAs first) ----------------------
        wq_sb = consts.tile([128, 1024], f16)
        wk_sb = consts.tile([128, 1024], f16)
        wv_sb = consts.tile([128, 1024], f16)
        for sb, dram in ((wq_sb, wq), (wk_sb, wk), (wv_sb, wv)):
            nc.sync.dma_start(
                out=sb.rearrange("p (t m) -> p t m", m=128),
                in_=dram.rearrange("(t p) m -> p t m", p=128),
            )
        wo_sb = consts.tile([128, 1024], f16)
        nc.sync.dma_start(out=wo_sb, in_=wo)
        tri_sb = consts.tile([128, 128], f16)
        nc.sync.dma_start(out=tri_sb, in_=tri)
        bcw_sb = consts.tile([128, 256], f16)
        nc.sync.dma_start(out=bcw_sb, in_=bcw)
        zero_sb = consts.tile([128, 128], f16)
        nc.vector.memset(zero_sb, 0.0)
        # preload the exp activation table while the weight DMAs run, so
        # the ~2.7us ACT_TABLE_LOAD is off the first chunk's critical path
        pre = smallp.tile([1, 64], f32, tag="pre")
        nc.scalar.activation(pre, zero_sb[0:1, 0:64], AF.Exp, scale=1.0)

        qT_sb = persist.tile([128, S], f16)  # RoPE'd q, [dk(2 heads), s]
        kT_sb = persist.tile([128, S], f16)
        # v_sb per k-tile t (VS=256 cols):
        #   cols 0:128  = [vA(64) | 1A(col 64) | 0(63)]   -> pvA + denA@p64
        #   cols 128:256 = [1B(col 128) | 0(63) | vB(64)] -> denB@p0 + pvB
        v_sb = persist.tile([128, 32 * VS], f16)
        nc.vector.memset(v_sb, 0.0)
        vv = v_sb.rearrange("p (t c) -> p t c", c=VS)
        for t in range(32):
            nc.sync.dma_start(out=vv[:, t, 64:65], in_=ones)
            nc.sync.dma_start(out=vv[:, t, 128:129], in_=ones)

        def rope(ps, dst, j):
            """dst = ps * cos + swap(ps) * sin  (chunk j), fp16 datapath."""
            cs, sn = pstate[("trig", j)]
            t0 = ropep.tile([128, CH], f16, tag="t0")
            nc.vector.tensor_copy(t0, ps)
            sw = ropep.tile([128, CH], f16, tag="sw")
            nc.vector.stream_shuffle(sw, t0, SWAP_MASK)
            t1 = ropep.tile([128, CH], f16, tag="t1")
            nc.vector.tensor_tensor(t1, t0, cs, OP.mult)
            t2 = ropep.tile([128, CH], f16, tag="t2")
            nc.vector.tensor_tensor(t2, sw, sn, OP.mult)
            nc.vector.tensor_tensor(dst, t1, t2, OP.add)

        def proj_u1(j):
            """q projection + RoPE(q)."""
            jsl = slice(j * CH, (j + 1) * CH)
            if ("xt", j) not in pstate:
                fetch_chunk(j)
            xt = pstate[("xt", j)]
            psq = ps_misc.tile([128, CH], f32, tag="m")
            for t in range(8):
                nc.tensor.matmul(psq, wq_sb[:, t * 128:(t + 1) * 128],
                                 xt[:, t * CH:(t + 1) * CH],
                                 start=(t == 0), stop=(t == 7))
            rope(psq, qT_sb[:, jsl], j)

        def proj_u2(j):
            """k projection + RoPE(k)."""
            jsl = slice(j * CH, (j + 1) * CH)
            xt = pstate[("xt", j)]
            psk = ps_misc.tile([128, CH], f32, tag="m")
            for t in range(8):
                nc.tensor.matmul(psk, wk_sb[:, t * 128:(t + 1) * 128],
                                 xt[:, t * CH:(t + 1) * CH],
                                 start=(t == 0), stop=(t == 7))
            rope(psk, kT_sb[:, jsl], j)
            del pstate[("trig", j)]

        def proj_u3(j):
            """v projection, directly in [k, d] layout: stationary = x
            s-subtile, moving = Wv -> out[s, vd]. 4 s-subtiles x 8 xd."""
            xt = pstate.pop(("xt", j))
            psv = ps_misc.tile([128, CH], f32, tag="m")
            # explicit full-width zeroing MM: safe under both per-element
            # and whole-bank has_written-clear semantics.
            nc.tensor.matmul(psv, zero_sb, wv_sb[:, 0:CH],
                             start=True, stop=False, skip_group_check=True)
            for u in range(4):
                for t in range(8):
                    nc.tensor.matmul(
                        psv[:, u * 128:(u + 1) * 128],
                        xt[:, t * CH + u * 128:t * CH + (u + 1) * 128],
                        wv_sb[:, t * 128:(t + 1) * 128],
                        start=False, stop=(t == 7),
                        skip_group_check=True)
            # scatter into v_sb: head A dims -> cols t*VS+0:64,
            # head B dims -> cols t*VS+192:256
            t0 = 4 * j
            nc.vector.tensor_copy(
                vv[:, t0:t0 + 4, 0:64],
                psv.rearrange("p (u c) -> p u c", c=128)[:, :, 0:64])
            nc.vector.tensor_copy(
                vv[:, t0:t0 + 4, 192:256],
                psv.rearrange("p (u c) -> p u c", c=128)[:, :, 64:128])

        def attn_scores(j, t):
            """Scores for k-tile t against chunk j's queries (heads A||B
            row-tiled, packed in one [128, 2CH] PSUM tile), then one exp
            ACTIVATE into an fp16 SBUF tile. Idempotent per tile."""
            if ("e", t) in pstate:
                return
            jsl = slice(j * CH, (j + 1) * CH)
            off = max(0, 128 * (t - 4 * j))
            diag = t >= 4 * j
            ksl = slice(t * 128, (t + 1) * 128)
            sAB = ps_scores.tile([128, 2 * CH], f32, tag="s")
            nc.tensor.matmul(sAB[:, off:CH], kT_sb[0:64, ksl],
                             qT_sb[0:64, jsl][:, off:CH],
                             start=True, stop=True)
            nc.tensor.matmul(sAB[:, CH + off:2 * CH], kT_sb[64:128, ksl],
                             qT_sb[64:128, jsl][:, off:CH],
                             start=True, stop=True)
            eAB = expp.tile([128, 2 * CH], f16, tag="e")
            if not diag:
                nc.scalar.activation(eAB, sAB, AF.Exp, scale=0.125)
            else:
                # two partial ACTIVATEs: only read PSUM regions the score
                # matmuls actually wrote (reading never-written PSUM is
                # unreliable on hardware).
                nc.scalar.activation(eAB[:, off:CH], sAB[:, off:CH],
                                     AF.Exp, scale=0.125)
                nc.scalar.activation(eAB[:, CH + off:2 * CH],
                                     sAB[:, CH + off:2 * CH],
                                     AF.Exp, scale=0.125)
                for hb in (0, CH):
                    tm = slice(hb + off, hb + off + 128)
                    nc.vector.tensor_tensor(eAB[:, tm], eAB[:, tm], tri_sb,
                                            OP.mult)
            pstate[("e", t)] = eAB

        def attn_pv(j, t, last):
            """PV for k-tile t into chunk j's pvA/pvB accumulators."""
            eAB = pstate.pop(("e", t))
            pvA, pvB = pstate[("pv", j)]
            off = max(0, 128 * (t - 4 * j))
            first = t == 0
            nc.tensor.matmul(pvA[:, off:CH], v_sb[:, t * VS:t * VS + 128],
                             eAB[:, off:CH],
                             start=first, stop=last, skip_group_check=True)
            nc.tensor.matmul(pvB[:, off:CH],
                             v_sb[:, t * VS + 128:t * VS + 256],
                             eAB[:, CH + off:2 * CH],
                             start=first, stop=last, skip_group_check=True)

        def fin_norm(j):
            """Normalize chunk j's pv into pv_sb fp16 [d(A 0:64, B 64:128), q].

            ds casts (pv/256 fp16) both release the pv psum banks AND feed
            the normalize multiplies; selector matmuls broadcast the den
            rows; full-tile reciprocal gives 256/den, so ds*bc = pv/den
            exactly. All ops full-128-partition (single-partition custom
            DVE ops on PSUM crash the device intermittently).
            """
            pvA, pvB = pstate.pop(("pv", j))
            dsA = smallp.tile([128, CH], f16, tag="dsA")
            nc.vector.tensor_scalar(dsA, pvA, 1.0 / 256, None, OP.mult)
            dsB = smallp.tile([128, CH], f16, tag="dsB")
            nc.vector.tensor_scalar(dsB, pvB, 1.0 / 256, None, OP.mult)
            bc_ps = ps_misc.tile([128, CH], f32, tag="m")
            nc.tensor.matmul(bc_ps, bcw_sb[:, 0:128], dsA,
                             start=True, stop=False, skip_group_check=True)
            nc.tensor.matmul(bc_ps, bcw_sb[:, 128:256], dsB,
                             start=False, stop=True, skip_group_check=True)
            bc_sb = bcp.tile([128, CH], f32, tag="bc")
            nc.vector.reciprocal_approx_fast(out=bc_sb, in_=bc_ps)
            pv_sb = pvsbp.tile([128, CH], f16, tag="pv")
            nc.vector.tensor_tensor(pv_sb[0:64, :], dsA[0:64, :],
                                    bc_sb[0:64, :], OP.mult)
            nc.vector.tensor_tensor(pv_sb[64:128, :], dsB[64:128, :],
                                    bc_sb[64:128, :], OP.mult)
            pstate[("pvn", j)] = pv_sb

        def fin_out(j, st):
            """Out-projection + store for 128 output rows of chunk j.
            256-col quarters double-buffered in one PSUM bank so the PE
            never waits on the previous quarter's cast."""
            pv_sb = pstate[("pvn", j)]
            stg = stagep.tile([128, 1024], f16, tag="stg")
            for oc in range(4):
                ops = ps_out.tile([128, 256], f32, tag="o")
                nc.tensor.matmul(ops, pv_sb[:, st * 128:(st + 1) * 128],
                                 wo_sb[:, oc * 256:(oc + 1) * 256],
                                 start=True, stop=True)
                nc.vector.tensor_copy(stg[:, oc * 256:(oc + 1) * 256], ops)
            if st == 3:
                del pstate[("pvn", j)]
            r0 = j * CH + st * 128
            nc.sync.dma_start(out=out[r0:r0 + 128, :], in_=stg)

        # ---- main loop -------------------------------------------------
        for u in (proj_u1, proj_u2, proj_u3):
            u(0)
        for j in range(nch):
            if j + 1 < nch:
                fetch_chunk(j + 1)   # DMA lead time before proj units run
            pvA = ps_pv.tile([128, CH], f32, tag="pvA")
            pvB = ps_pv.tile([128, CH], f32, tag="pvB")
            pstate[("pv", j)] = (pvA, pvB)
            tiles = list(range(4 * j + 4))
            units = []
            if j + 1 < nch:
                units.append(lambda jj=j + 1: proj_u1(jj))
                units.append(lambda jj=j + 1: proj_u2(jj))
            if j > 0:
                for st in range(4):
                    units.append(lambda jj=j - 1, ss=st: fin_out(jj, ss))
            if j + 1 < nch:
                units.append(lambda jj=j + 1: proj_u3(jj))
            ng = len(tiles)
            done = 0
            attn_scores(j, 0)
            for ti, t in enumerate(tiles):
                if t + 1 < ng:
                    attn_scores(j, t + 1)
                attn_pv(j, t, last=(t == ng - 1))
                want = (ti + 1) * len(units) // ng
                while done < want:
                    units[done]()
                    done += 1
            # normalize as soon as accumulation is complete: the ds casts
            # free the pv psum banks for the next chunk.
            fin_norm(j)
            # pre-emit the next chunk's first two score tiles so ACT rolls
            # into the next chunk's exp stream with no gap.
            if j + 1 < nch:
                attn_scores(j + 1, 0)
                attn_scores(j + 1, 1)

        for st in range(4):
            fin_out(nch - 1, st)


def _build(nch=NCH):
    import concourse.mybir as mybir
    import concourse.tile as tile
    from concourse import bacc

    f32 = mybir.dt.float32
    f16 = mybir.dt.float16
    nc = bacc.Bacc("TRN2", target_bir_lowering=False, debug=False,
                   num_devices=NCORES)
    aps = {}
    for name, shape in (
        ("xT", [D, S]), ("wq", [D, 128]), ("wk", [D, 128]), ("wv", [D, 128]),
        ("wo", [128, D]), ("tri", [128, 128]), ("bcw", [128, 256]),
        ("ones", [128, 1]), ("cos", [128, S]), ("sin", [128, S]),
    ):
        aps[name] = nc.dram_tensor(name, shape, f16, kind="ExternalInput").ap()
    out_ap = nc.dram_tensor("out", [S, D], f16, kind="ExternalOutput").ap()

    with tile.TileContext(nc) as tc:
        _emit(tc, out_ap, aps["xT"], aps["wq"], aps["wk"], aps["wv"],
              aps["wo"], aps["cos"], aps["sin"], aps["tri"], aps["bcw"],
              aps["ones"], nch=nch)
    nc.compile()
    return nc


def kernel(x, Wq, Wk, Wv, Wo):
    from concourse.bass_utils import run_bass_kernel_spmd

    if "nc" not in _CACHE:
        _CACHE["nc"] = _build()
    nc = _CACHE["nc"]

    in_maps = _host_prep(x, Wq, Wk, Wv, Wo)
    res = run_bass_kernel_spmd(nc, in_maps, core_ids=list(range(NCORES)))
    acc = np.zeros((S, D), dtype=np.float64)
    for r in res.results:
        acc += r["out"].astype(np.float64)
    return acc.astype(np.float32).reshape(1, S, D)


# revision 22
# speedup vs baseline: 1.3016x; 1.3016x over previous
"""Multi-head self-attention (RoPE, causal) Trainium2 kernel, v3.

Tensor-parallel over heads: 16 heads / 8 cores = 2 heads per core
(Megatron-style: Wq/Wk/Wv sharded on output dim, Wo on input dim).
Each core computes a full [S, D] partial of the output projection;
the host sums the 8 partials.

v3 changes vs v2:
- PV orientation flipped: stationary = [vA|1A|0] / [1B|0|vB] zero-padded
  128-col blocks (per k-tile), moving = the exp tile (512 q cols). Kills
  the LDWEIGHTS-bound PV of v2 (~90us -> ~61us PE) and produces pv in
  [d, q] layout with softmax denominators embedded at partitions 64 (A)
  and 0 (B) -- no ones-columns in the moving operand, no PV psum
  zero-matmuls, no attention transposes.
- v projection emitted directly in [k, d] layout (stationary = x tile,
  moving = Wv) -- kills the v transposes.
- Normalization: reciprocal of den rows -> tiny ones-stationary matmul
  broadcasts them across partitions -> one tensor_tensor per head per
  chunk normalizes pv before a single-stationary out-projection.
- One ACTIVATE per tile-instance (diag tiles exp a harmless garbage
  region that PV never reads).
- fp16 RoPE datapath and fp16 trig tables (halves trig DMA).
"""

import numpy as np

S = 4096
D = 1024
DK = 64
NCORES = 8
THETA = 10000.0
CH = 512          # sequence chunk (scores/PV moving free dim)
NCH = S // CH     # 8 chunks
VS = 256          # v_sb cols per k-tile: [vA|1A|0(63)] [1B|0(63)|vB]

_CACHE = {}


# ---------------------------------------------------------------------------
# host-side layout helpers
# ---------------------------------------------------------------------------

def _rope_perm64():
    """Permutation of a head's 64 dims so RoPE pairs line up for a
    32-lane stream_shuffle: quadrant q (32 partitions) holds pairs
    16q..16q+15 as [evens(16) | odds(16)]."""
    perm = np.zeros(64, np.int64)
    for d in range(64):
        j, odd = d // 2, d % 2
        pos = 32 * (j // 16) + 16 * odd + (j % 16)
        perm[pos] = d
    return perm


def _trig_tables():
    # partition p: pair index = 16*((p//32)%2) + p%16 ; odd slot if p%32 >= 16
    p = np.arange(128)
    pair = 16 * ((p // 32) % 2) + (p % 16)
    odd = (p % 32) >= 16
    inv_freq = THETA ** (-2.0 * pair / DK)           # [128]
    pos = np.arange(S, dtype=np.float64)
    ang = pos[None, :] * inv_freq[:, None]           # [128, S]
    cos = np.cos(ang).astype(np.float16)
    sin = (np.where(odd[:, None], 1.0, -1.0) * np.sin(ang)).astype(np.float16)
    return cos, sin


def _host_prep(x, Wq, Wk, Wv, Wo):
    x = np.asarray(x, dtype=np.float32).reshape(S, D)
    Wq = np.asarray(Wq, dtype=np.float32)
    Wk = np.asarray(Wk, dtype=np.float32)
    Wv = np.asarray(Wv, dtype=np.float32)
    Wo = np.asarray(Wo, dtype=np.float32)

    xT = np.ascontiguousarray(x.T).astype(np.float16)          # [D, S]
    cos, sin = _trig_tables()
    tri = (np.arange(128)[None, :] >= np.arange(128)[:, None])
    tri = tri.astype(np.float16)

    # den-broadcast selector matmul weights [128, 256]:
    # A-part [:, 0:128]: row 64 (denA) -> out partitions 0:64;
    # B-part [:, 128:256]: row 0 (denB) -> out partitions 64:128.
    bcw = np.zeros((128, 256), np.float32)
    bcw[64, 0:64] = 1.0
    bcw[0, 128 + 64:256] = 1.0
    bcw = bcw.astype(np.float16)

    perm = _rope_perm64()
    in_maps = []
    for c in range(NCORES):
        hA, hB = 2 * c, 2 * c + 1
        rows_qk = np.concatenate([64 * hA + perm, 64 * hB + perm])
        rows_v = np.arange(128 * c, 128 * c + 128)
        wq_c = np.ascontiguousarray(Wq[rows_qk, :].T).astype(np.float16)
        wk_c = np.ascontiguousarray(Wk[rows_qk, :].T).astype(np.float16)
        wv_c = np.ascontiguousarray(Wv[rows_v, :].T).astype(np.float16)
        wo_c = np.ascontiguousarray(Wo[:, rows_v].T).astype(np.float16)
        in_maps.append({
            "xT": xT, "wq": wq_c, "wk": wk_c, "wv": wv_c, "wo": wo_c,
            "cos": cos, "sin": sin, "tri": tri, "bcw": bcw,
            "ones": np.ones((128, 64), np.float16),
        })
    return in_maps


# ---------------------------------------------------------------------------
# device program
# ---------------------------------------------------------------------------

def _emit(tc, out, xT, wq, wk, wv, wo, cos, sin, tri, bcw, ones, nch=NCH):
    import concourse.mybir as mybir

    nc = tc.nc
    f32 = mybir.dt.float32
    f16 = mybir.dt.float16
    AF = mybir.ActivationFunctionType
    OP = mybir.AluOpType
    SWAP_MASK = [(i + 16) % 32 for i in range(32)]

    with (
        tc.tile_pool(name="consts", bufs=1) as consts,
        tc.tile_pool(name="persist", bufs=1) as persist,
        tc.tile_pool(name="xtp", bufs=2) as xtp,
        tc.tile_pool(name="rope", bufs=2) as ropep,
        tc.tile_pool(name="trig", bufs=2) as trigp,
        tc.tile_pool(name="expp", bufs=4) as expp,
        tc.tile_pool(name="small", bufs=4) as smallp,
        tc.tile_pool(name="pvsb", bufs=2) as pvsbp,
        tc.tile_pool(name="bcp", bufs=2) as bcp,
        tc.tile_pool(name="stagep", bufs=3) as stagep,
        tc.tile_pool(name="ps_s", bufs=2, space="PSUM") as ps_scores,
        tc.tile_pool(name="ps_pv", bufs=1, space="PSUM") as ps_pv,
        tc.tile_pool(name="ps_o", bufs=1, space="PSUM") as ps_out,
        tc.tile_pool(name="ps_m", bufs=1, space="PSUM") as ps_misc,
    ):
        pstate = {}

        def fetch_chunk(j):
            """Issue chunk j's x and trig DMAs."""
            if ("xt", j) in pstate:
                return
            jsl = slice(j * CH, (j + 1) * CH)
            xt = xtp.tile([128, 8 * CH], f16, tag="xt")
            pstate[("xt", j)] = xt
            nc.sync.dma_start(
                out=xt.rearrange("p (t s) -> p t s", s=CH),
                in_=xT[:, jsl].rearrange("(t p) s -> p t s", p=128),
            )
            cs = trigp.tile([128, CH], f16, tag="cs")
            nc.sync.dma_start(out=cs, in_=cos[:, jsl])
            sn = trigp.tile([128, CH], f16, tag="sn")
            nc.sync.dma_start(out=sn, in_=sin[:, jsl])
            pstate[("trig", j)] = (cs, sn)

        # ---- constants (critical-path DMAs first) ----------------------
        wq_sb = consts.tile([128, 1024], f16)
        wk_sb = consts.tile([128, 1024], f16)
        wv_sb = consts.tile([128, 1024], f16)
        for sb, dram in ((wq_sb, wq), (wk_sb, wk), (wv_sb, wv)):
            nc.sync.dma_start(
                out=sb.rearrange("p (t m) -> p t m", m=128),
                in_=dram.rearrange("(t p) m -> p t m", p=128),
            )
        # chunk 0's x/trig next on the queue -- ahead of consts that are
        # first needed later (tri at first diag exp, ones at first PV,
        # wo/bcw at the first chunk's finish work).
        fetch_chunk(0)
        tri_sb = consts.tile([128, 128], f16)
        nc.sync.dma_start(out=tri_sb, in_=tri)
        v_sb = persist.tile([128, 32 * VS], f16)
        nc.vector.memset(v_sb, 0.0)
        vv = v_sb.rearrange("p (t c) -> p t c", c=VS)
        ones32 = ones.rearrange("p (t o) -> p t o", o=2)[:, 0:32, :]
        nc.sync.dma_start(out=vv[:, :, 64:65], in_=ones32[:, :, 0:1])
        nc.sync.dma_start(out=vv[:, :, 128:129], in_=ones32[:, :, 1:2])
        wo_sb = consts.tile([128, 1024], f16)
        nc.sync.dma_start(out=wo_sb, in_=wo)
        bcw_sb = consts.tile([128, 256], f16)
        nc.sync.dma_start(out=bcw_sb, in_=bcw)
        zero_sb = consts.tile([128, 128], f16)
        nc.vector.memset(zero_sb, 0.0)
        # preload the exp activation table while the weight DMAs run, so
        # the ~2.7us ACT_TABLE_LOAD is off the first chunk's critical path
        pre = smallp.tile([1, 64], f32, tag="pre")
        nc.scalar.activation(pre, zero_sb[0:1, 0:64], AF.Exp, scale=1.0)

        qT_sb = persist.tile([128, S], f16)  # RoPE'd q, [dk(2 heads), s]
        kT_sb = persist.tile([128, S], f16)

        def rope(ps, dst, j):
            """dst = ps * cos + swap(ps) * sin  (chunk j), fp16 datapath."""
            cs, sn = pstate[("trig", j)]
            t0 = ropep.tile([128, CH], f16, tag="t0")
            nc.vector.tensor_copy(t0, ps)
            sw = ropep.tile([128, CH], f16, tag="sw")
            nc.vector.stream_shuffle(sw, t0, SWAP_MASK)
            t1 = ropep.tile([128, CH], f16, tag="t1")
            nc.vector.tensor_tensor(t1, t0, cs, OP.mult)
            t2 = ropep.tile([128, CH], f16, tag="t2")
            nc.vector.tensor_tensor(t2, sw, sn, OP.mult)
            nc.vector.tensor_tensor(dst, t1, t2, OP.add)

        def proj_u1(j):
            """q projection + RoPE(q)."""
            jsl = slice(j * CH, (j + 1) * CH)
            if ("xt", j) not in pstate:
                fetch_chunk(j)
            xt = pstate[("xt", j)]
            psq = ps_misc.tile([128, CH], f32, tag="m")
            for t in range(8):
                nc.tensor.matmul(psq, wq_sb[:, t * 128:(t + 1) * 128],
                                 xt[:, t * CH:(t + 1) * CH],
                                 start=(t == 0), stop=(t == 7))
            rope(psq, qT_sb[:, jsl], j)

        def proj_u2(j):
            """k projection + RoPE(k)."""
            jsl = slice(j * CH, (j + 1) * CH)
            xt = pstate[("xt", j)]
            psk = ps_misc.tile([128, CH], f32, tag="m")
            for t in range(8):
                nc.tensor.matmul(psk, wk_sb[:, t * 128:(t + 1) * 128],
                                 xt[:, t * CH:(t + 1) * CH],
                                 start=(t == 0), stop=(t == 7))
            rope(psk, kT_sb[:, jsl], j)
            del pstate[("trig", j)]

        def proj_u3(j):
            """v projection, directly in [k, d] layout: stationary = x
            s-subtile, moving = Wv -> out[s, vd]. 4 s-subtiles x 8 xd."""
            xt = pstate.pop(("xt", j))
            psv = ps_misc.tile([128, CH], f32, tag="m")
            # explicit full-width zeroing MM: safe under both per-element
            # and whole-bank has_written-clear semantics.
            nc.tensor.matmul(psv, zero_sb, wv_sb[:, 0:CH],
                             start=True, stop=False, skip_group_check=True)
            for u in range(4):
                for t in range(8):
                    nc.tensor.matmul(
                        psv[:, u * 128:(u + 1) * 128],
                        xt[:, t * CH + u * 128:t * CH + (u + 1) * 128],
                        wv_sb[:, t * 128:(t + 1) * 128],
                        start=False, stop=(t == 7),
                        skip_group_check=True)
            # scatter into v_sb: head A dims -> cols t*VS+0:64,
            # head B dims -> cols t*VS+192:256
            t0 = 4 * j
            nc.vector.tensor_copy(
                vv[:, t0:t0 + 4, 0:64],
                psv.rearrange("p (u c) -> p u c", c=128)[:, :, 0:64])
            nc.vector.tensor_copy(
                vv[:, t0:t0 + 4, 192:256],
                psv.rearrange("p (u c) -> p u c", c=128)[:, :, 64:128])

        def attn_scores(j, t):
            """Scores for k-tile t against chunk j's queries (heads A||B
            row-tiled, packed in one [128, 2CH] PSUM tile), then one exp
            ACTIVATE into an fp16 SBUF tile. Idempotent per tile."""
            if ("e", t) in pstate:
                return
            jsl = slice(j * CH, (j + 1) * CH)
            off = max(0, 128 * (t - 4 * j))
            diag = t >= 4 * j
            ksl = slice(t * 128, (t + 1) * 128)
            sAB = ps_scores.tile([128, 2 * CH], f32, tag="s")
            nc.tensor.matmul(sAB[:, off:CH], kT_sb[0:64, ksl],
                             qT_sb[0:64, jsl][:, off:CH],
                             start=True, stop=True)
            nc.tensor.matmul(sAB[:, CH + off:2 * CH], kT_sb[64:128, ksl],
                             qT_sb[64:128, jsl][:, off:CH],
                             start=True, stop=True)
            eAB = expp.tile([128, 2 * CH], f16, tag="e")
            if not diag:
                nc.scalar.activation(eAB, sAB, AF.Exp, scale=0.125)
            else:
                # two partial ACTIVATEs: only read PSUM regions the score
                # matmuls actually wrote (reading never-written PSUM is
                # unreliable on hardware).
                nc.scalar.activation(eAB[:, off:CH], sAB[:, off:CH],
                                     AF.Exp, scale=0.125)
                nc.scalar.activation(eAB[:, CH + off:2 * CH],
                                     sAB[:, CH + off:2 * CH],
                                     AF.Exp, scale=0.125)
                for hb in (0, CH):
                    tm = slice(hb + off, hb + off + 128)
                    nc.vector.tensor_tensor(eAB[:, tm], eAB[:, tm], tri_sb,
                                            OP.mult)
            pstate[("e", t)] = eAB

        def attn_pv(j, t, last):
            """PV for k-tile t into chunk j's pvA/pvB accumulators."""
            eAB = pstate.pop(("e", t))
            pvA, pvB = pstate[("pv", j)]
            off = max(0, 128 * (t - 4 * j))
            first = t == 0
            nc.tensor.matmul(pvA[:, off:CH], v_sb[:, t * VS:t * VS + 128],
                             eAB[:, off:CH],
                             start=first, stop=last, skip_group_check=True)
            nc.tensor.matmul(pvB[:, off:CH],
                             v_sb[:, t * VS + 128:t * VS + 256],
                             eAB[:, CH + off:2 * CH],
                             start=first, stop=last, skip_group_check=True)

        def fin_norm(j):
            """Normalize chunk j's pv into pv_sb fp16 [d(A 0:64, B 64:128), q].

            ds casts (pv/256 fp16) both release the pv psum banks AND feed
            the normalize multiplies; selector matmuls broadcast the den
            rows; full-tile reciprocal gives 256/den, so ds*bc = pv/den
            exactly. All ops full-128-partition (single-partition custom
            DVE ops on PSUM crash the device intermittently).
            """
            pvA, pvB = pstate.pop(("pv", j))
            dsA = smallp.tile([128, CH], f16, tag="dsA")
            nc.vector.tensor_scalar(dsA, pvA, 1.0 / 256, None, OP.mult)
            dsB = smallp.tile([128, CH], f16, tag="dsB")
            nc.vector.tensor_scalar(dsB, pvB, 1.0 / 256, None, OP.mult)
            bc_ps = ps_misc.tile([128, CH], f32, tag="m")
            nc.tensor.matmul(bc_ps, bcw_sb[:, 0:128], dsA,
                             start=True, stop=False, skip_group_check=True)
            nc.tensor.matmul(bc_ps, bcw_sb[:, 128:256], dsB,
                             start=False, stop=True, skip_group_check=True)
            bc_sb = bcp.tile([128, CH], f32, tag="bc")
            nc.vector.reciprocal_approx_fast(out=bc_sb, in_=bc_ps)
            pv_sb = pvsbp.tile([128, CH], f16, tag="pv")
            nc.vector.tensor_tensor(pv_sb[0:64, :], dsA[0:64, :],
                                    bc_sb[0:64, :], OP.mult)
            nc.vector.tensor_tensor(pv_sb[64:128, :], dsB[64:128, :],
                                    bc_sb[64:128, :], OP.mult)
            pstate[("pvn", j)] = pv_sb

        def fin_out(j, st):
            """Out-projection + store for 128 output rows of chunk j.
            256-col quarters double-buffered in one PSUM bank so the PE
            never waits on the previous quarter's cast."""
            pv_sb = pstate[("pvn", j)]
            stg = stagep.tile([128, 1024], f16, tag="stg")
            for oc in range(4):
                ops = ps_out.tile([128, 256], f32, tag="o")
                nc.tensor.matmul(ops, pv_sb[:, st * 128:(st + 1) * 128],
                                 wo_sb[:, oc * 256:(oc + 1) * 256],
                                 start=True, stop=True)
                nc.vector.tensor_copy(stg[:, oc * 256:(oc + 1) * 256], ops)
            if st == 3:
                del pstate[("pvn", j)]
            r0 = j * CH + st * 128
            nc.sync.dma_start(out=out[r0:r0 + 128, :], in_=stg)

        # ---- main loop -------------------------------------------------
        for u in (proj_u1, proj_u2, proj_u3):
            u(0)
        for j in range(nch):
            if j + 1 < nch:
                fetch_chunk(j + 1)   # DMA lead time before proj units run
            pvA = ps_pv.tile([128, CH], f32, tag="pvA")
            pvB = ps_pv.tile([128, CH], f32, tag="pvB")
            pstate[("pv", j)] = (pvA, pvB)
            tiles = list(range(4 * j + 4))
            units = []
            if j + 1 < nch:
                units.append(lambda jj=j + 1: proj_u1(jj))
                units.append(lambda jj=j + 1: proj_u2(jj))
            if j > 0:
                for st in range(4):
                    units.append(lambda jj=j - 1, ss=st: fin_out(jj, ss))
            if j + 1 < nch:
                units.append(lambda jj=j + 1: proj_u3(jj))
            ng = len(tiles)
            done = 0
            attn_scores(j, 0)
            for ti, t in enumerate(tiles):
                if t + 1 < ng:
                    attn_scores(j, t + 1)
                attn_pv(j, t, last=(t == ng - 1))
                want = (ti + 1) * len(units) // ng
                while done < want:
                    units[done]()
                    done += 1
            # normalize as soon as accumulation is complete: the ds casts
            # free the pv psum banks for the next chunk.
            fin_norm(j)
            # pre-emit the next chunk's first two score tiles so ACT rolls
            # into the next chunk's exp stream with no gap.
            if j + 1 < nch:
                attn_scores(j + 1, 0)
                attn_scores(j + 1, 1)

        for st in range(4):
            fin_out(nch - 1, st)


def _build(nch=NCH):
    import concourse.mybir as mybir
    import concourse.tile as tile
    from concourse import bacc

    f32 = mybir.dt.float32
    f16 = mybir.dt.float16
    nc = bacc.Bacc("TRN2", target_bir_lowering=False, debug=False,
                   num_devices=NCORES)
    aps = {}
    for name, shape in (
        ("xT", [D, S]), ("wq", [D, 128]), ("wk", [D, 128]), ("wv", [D, 128]),
        ("wo", [128, D]), ("tri", [128, 128]), ("bcw", [128, 256]),
        ("ones", [128, 64]), ("cos", [128, S]), ("sin", [128, S]),
    ):
        aps[name] = nc.dram_tensor(name, shape, f16, kind="ExternalInput").ap()
    out_ap = nc.dram_tensor("out", [S, D], f16, kind="ExternalOutput").ap()

    with tile.TileContext(nc) as tc:
        _emit(tc, out_ap, aps["xT"], aps["wq"], aps["wk"], aps["wv"],
              aps["wo"], aps["cos"], aps["sin"], aps["tri"], aps["bcw"],
              aps["ones"], nch=nch)
    nc.compile()
    return nc


def kernel(x, Wq, Wk, Wv, Wo):
    from concourse.bass_utils import run_bass_kernel_spmd

    if "nc" not in _CACHE:
        _CACHE["nc"] = _build()
    nc = _CACHE["nc"]

    in_maps = _host_prep(x, Wq, Wk, Wv, Wo)
    res = run_bass_kernel_spmd(nc, in_maps, core_ids=list(range(NCORES)))
    acc = np.zeros((S, D), dtype=np.float64)
    for r in res.results:
        acc += r["out"].astype(np.float64)
    return acc.astype(np.float32).reshape(1, S, D)
